# revision 31
# baseline (speedup 1.0000x reference)
"""Trainium2 Bass kernel for nn_Conv2d_ONI (1x1 conv with ONI-orthogonalized weight).

Strategy:
  - Data-parallel: shard x [32,64,128,128] over batch across 8 NeuronCores
    (4 images each); z/g/bias replicated; ONI (Newton-Schulz on 64x64)
    recomputed on every core (microscopic vs the conv).
  - The kernel is HBM-bound (per-core ~358-420 GB/s HBM shared by
    loads+stores).  To halve HBM traffic, x is cast to bf16 on the HOST
    before upload and the output is stored as bf16 and upcast on the host:
    8.4 MB in + 8.4 MB out per core instead of 33.6 MB total.
  - Per core, the 1x1 conv is a 64x64 channel matmul over 4*128*128
    positions.  Image pairs are stacked on SBUF partitions; the weight is
    packed BLOCK-DIAGONALLY into one [128,128] bf16 stationary tile so a
    single K=128 matmul instruction computes both images per 512-col
    slice.
  - ONI head is restructured to minimize the serial critical path before
    the conv can start:
      * push-through identity: w = sqrt2*diag(g)*v*poly(v^T v), so
        s1 = A^T A comes straight from matmul(lhsT=A, rhs=A) -- the PE
        transpose + PSUM copy drop off the head; the one remaining
        transpose (of v*g) runs DURING the Newton-Schulz loop.
      * b1 = 1.5I - 0.5*invn*s1 fused via a pre-scaled broadcast column.
      * loop PSUM->SBUF ops all on DVE (fewer cross-engine sem hops).
  - PSUM pools are scoped: the ONI pools close before the conv pool opens,
    so the conv gets 4 x 2-bank PSUM buffers -- DVE and ACT bias-add
    copies run concurrently on different banks and the PE never stalls
    more than 4 chunks ahead.
  - DMA: 8 loads + 8 stores of 1 MiB each; loads on the sync HWDGE ring
    (parm first, so ONI starts as early as possible), stores on the
    scalar/ACT ring.  All 16 SBUF tiles resident (no buffer-reuse stalls).
"""

import sys

for _p in ("/opt/trn_rl_repo",):
    if _p not in sys.path:
        sys.path.insert(0, _p)

import ml_dtypes
import numpy as np

import concourse.bass as bass  # noqa: F401  (needed for engine registration)
import concourse.mybir as mybir
import concourse.tile as tile
from concourse import bacc
from concourse.bass_utils import run_bass_kernel_spmd

F32 = mybir.dt.float32
BF16 = mybir.dt.bfloat16
FP16 = mybir.dt.float16
AL = mybir.AluOpType
SQRT2 = float(np.sqrt(2.0))

N_CORES = 8
N_FULL = 32           # full batch
NB = N_FULL // N_CORES  # images per core (4)
C = 64                # in = out channels
H = W = 128
HW = H * W            # 16384 positions per image
GR = 4096             # granule free size ([128, GR] bf16 tile = 1 MiB)
ONI_ITR = 5
PCOLS = 66            # packed parm tensor columns (z | bias | g)


def _build():
    nc = bacc.Bacc("TRN2", target_bir_lowering=False, debug=False)

    x_h = nc.dram_tensor("x", [NB, C, H, W], BF16, kind="ExternalInput")
    parm_h = nc.dram_tensor("parm", [2 * C, PCOLS], F32, kind="ExternalInput")
    eye_h = nc.dram_tensor("eye", [C, C], F32, kind="ExternalInput")
    y_h = nc.dram_tensor("out", [NB, C, H, W], BF16, kind="ExternalOutput")

    # [NB, C, H, W] -> [NB/2, 128, HW]: image pairs stacked on partitions.
    xv = x_h[:].rearrange("(n2 two) c h w -> n2 (two c) (h w)", two=2)
    yv = y_h[:].rearrange("(n2 two) c h w -> n2 (two c) (h w)", two=2)

    with tile.TileContext(nc) as tc:
        with tc.tile_pool(name="consts", bufs=1) as sb, \
             tc.tile_pool(name="nsit", bufs=2) as it, \
             tc.tile_pool(name="xp", bufs=8) as xp, \
             tc.tile_pool(name="op", bufs=8) as op:

            # slim parm load goes first on the sync ring so the ONI serial
            # chain starts as early as possible (34 KB: transfer is
            # receipt-latency dominated); the x granule loads flood FIFO
            # behind parm (issued up front so their dispatches all land
            # before any store-wait can block the ring).
            parm_sb = sb.tile([2 * C, PCOLS], F32)
            nc.sync.dma_start(out=parm_sb, in_=parm_h[:])
            z_sb = parm_sb[0:C, 0:C]
            bias_sb = parm_sb[:, C : C + 1]        # [128,1]
            g_sb = parm_sb[0:C, C + 1 : C + 2]     # [C,1] g column

            xts = []
            for n2 in range(NB // 2):
                for gi in range(HW // GR):
                    xt = xp.tile([2 * C, GR], BF16, tag="xt",
                                 name=f"xt{n2}_{gi}")
                    nc.sync.dma_start(out=xt,
                                      in_=xv[n2, :, gi * GR : (gi + 1) * GR])
                    xts.append(xt)

            # gpsimd: memsets first (cheap), then the eye DMA (SWDGE
            # descriptor gen takes ~1us and would delay the memsets).
            warm_sb = sb.tile([C, C], FP16)
            nc.gpsimd.memset(warm_sb, 0.25)
            onesr_sb = sb.tile([1, C], F32)
            nc.gpsimd.memset(onesr_sb, 1.0)
            scr_sb = sb.tile([1, 4], F32)
            nc.gpsimd.memset(scr_sb, 1.0)
            eye_sb = sb.tile([C, C], F32)
            nc.gpsimd.dma_start(out=eye_sb, in_=eye_h[:])
            eye15h_sb = sb.tile([C, C], FP16)
            nc.vector.tensor_scalar_mul(eye15h_sb, eye_sb, 1.5)

            wT_sb = sb.tile([2 * C, 2 * C], BF16)
            nc.vector.memset(wT_sb, 0.0)

            # preload the ACT tables (Square/Sqrt) on scratch data during
            # the idle window while parm is still in flight -- a lazy
            # mid-chain ACT_TABLE_LOAD costs 1.3us on the crit path.
            nc.scalar.activation(out=scr_sb[:, 1:2], in_=scr_sb[:, 0:1],
                                 func=mybir.ActivationFunctionType.Square)
            nc.scalar.activation(out=scr_sb[:, 3:4], in_=scr_sb[:, 0:1],
                                 func=mybir.ActivationFunctionType.Sqrt)

            with tc.tile_pool(name="onips", bufs=3, space="PSUM") as psp, \
                 tc.tile_pool(name="wps", bufs=1, space="PSUM") as wpsp:

                # ---- ONI: weight = sqrt2*diag(g) * v * NS(v^T v) ----
                # A = C*z - rowsum (row centering; NS input self-normalizes
                # so the C* scaling cancels exactly through invn/rs).
                # The NS loop runs in fp16 (values are O(1); 1-pass
                # matmuls, 8x the mantissa of bf16 -- end-to-end error is
                # indistinguishable from the f32 loop given bf16 x/out).
                # dummy warm-up matmuls: one as soon as PE wakes, then one
                # per early x granule as it lands (~8.5/11us) -- spreads
                # PE activity across the serial head so the HAM clock gate
                # stays at 8/8 (idle >3.4us re-throttles to 1.2 GHz).
                warm_ps = wpsp.tile([C, C], F32, tag="warm")
                nc.tensor.matmul(warm_ps, warm_sb, warm_sb,
                                 start=True, stop=True)
                for wi in range(2):
                    warm_ps = wpsp.tile([C, C], F32, tag="warm")
                    nc.tensor.matmul(warm_ps, xts[wi][0:C, 0:C],
                                     xts[wi][0:C, 0:C],
                                     start=True, stop=True)

                rowsum = sb.tile([C, 1], F32)
                nc.vector.reduce_sum(rowsum, z_sb, axis=mybir.AxisListType.X)
                zc_sb = sb.tile([C, C], FP16)
                nc.vector.tensor_scalar(zc_sb, z_sb, float(C), rowsum,
                                        op0=AL.mult, op1=AL.subtract)

                # s1 = A^T A: A is its own lhsT -- no transpose needed.
                s1_ps = psp.tile([C, C], F32, tag="ps")
                nc.tensor.matmul(s1_ps, zc_sb, zc_sb, start=True, stop=True)

                # fro2 = sum(s1^2): ACT square+row-accumulate from PSUM,
                # then cross-partition matmul with a ones column.
                sq_sb = sb.tile([C, C], F32)
                colsq = sb.tile([C, 1], F32)
                nc.scalar.activation(out=sq_sb, in_=s1_ps,
                                     func=mybir.ActivationFunctionType.Square,
                                     accum_out=colsq)
                onesc_sb = sb.tile([C, 1], F32)
                nc.gpsimd.memset(onesc_sb, 1.0)
                fro2_ps = psp.tile([1, 1], F32, tag="ps")
                nc.tensor.matmul(fro2_ps, colsq, onesc_sb, start=True,
                                 stop=True)

                # invn = sqrt(1/fro2) = 1/||s1||_F, broadcast immediately
                # (s and b1 only need invn); rs*sqrt2 = sqrt(2*invn) and
                # its broadcast only feed vg -- off the critical path.
                rin_sb = sb.tile([1, 1], F32)
                nc.vector.reciprocal(rin_sb, fro2_ps)
                scal2 = sb.tile([1, 2], F32)
                nc.scalar.activation(out=scal2[:, 0:1], in_=rin_sb,
                                     func=mybir.ActivationFunctionType.Sqrt)
                bc_ps = psp.tile([C, 1], F32, tag="bc0", bufs=1)
                nc.tensor.matmul(bc_ps, onesr_sb, scal2[:, 0:1], start=True,
                                 stop=True)
                nc.scalar.activation(out=scal2[:, 1:2], in_=scal2[:, 0:1],
                                     func=mybir.ActivationFunctionType.Sqrt,
                                     scale=2.0)
                bc1_ps = psp.tile([C, 1], F32, tag="bc1", bufs=1)
                nc.tensor.matmul(bc1_ps, onesr_sb, scal2[:, 1:2], start=True,
                                 stop=True)

                # s = s1*invn (straight from PSUM); b1 = eye15 - 0.5*s
                s_sb = sb.tile([C, C], FP16)
                nc.vector.tensor_scalar_mul(s_sb, s1_ps, bc_ps[:, 0:1])
                b_sb = sb.tile([C, C], FP16)
                nc.vector.scalar_tensor_tensor(
                    out=b_sb, in0=s_sb, scalar=-0.5, in1=eye15h_sb,
                    op0=AL.mult, op1=AL.add,
                )

                # vg = diag(g)*A*(sqrt2*rs); vgT via PE transpose -- runs
                # DURING the loop (PE is mostly idle), off the crit path.
                vg_sb = sb.tile([C, C], F32)
                nc.vector.tensor_scalar(vg_sb, zc_sb, g_sb, bc1_ps[:, 0:1],
                                        op0=AL.mult, op1=AL.mult)
                vgT_ps = wpsp.tile([C, C], F32, tag="vgt")
                nc.tensor.transpose(vgT_ps, vg_sb, eye_sb)
                vgT_sb = sb.tile([C, C], FP16)
                nc.vector.tensor_copy(vgT_sb, vgT_ps)

                # b <- 1.5 b - 0.5 (b@b)(b@s); ph on ACT parallel with the
                # q copy on DVE; the 1.5b term rides the r PSUM
                # accumulation group (stationary 1.5I), so b_new is a
                # plain cast-copy -- no mixed-dtype op, no extra link.
                for _ in range(1, ONI_ITR):
                    p_ps = psp.tile([C, C], F32, tag="ps")
                    nc.tensor.matmul(p_ps, b_sb, b_sb, start=True, stop=True)
                    q_ps = psp.tile([C, C], F32, tag="ps")
                    nc.tensor.matmul(q_ps, b_sb, s_sb, start=True, stop=True)
                    ph_sb = it.tile([C, C], FP16, tag="ph")
                    nc.scalar.mul(ph_sb, p_ps, -0.5)
                    q_sb = it.tile([C, C], FP16, tag="q")
                    nc.vector.tensor_copy(q_sb, q_ps)
                    r_ps = psp.tile([C, C], F32, tag="ps")
                    nc.tensor.matmul(r_ps, ph_sb, q_sb, start=True,
                                     stop=False)
                    nc.tensor.matmul(r_ps, eye15h_sb, b_sb, start=False,
                                     stop=True)
                    b_new = it.tile([C, C], FP16, tag="b")
                    nc.vector.tensor_copy(b_new, r_ps)
                    b_sb = b_new

                # wT = b' @ vgT on BOTH partition halves via one matmul:
                # stationary b2 = [b'|b'] (128 cols) -> out [128, C]; the
                # two block-diag quadrant copies then run DVE || ACT.
                b2_sb = sb.tile([C, 2 * C], FP16)
                nc.vector.tensor_copy(b2_sb[:, 0:C], b_sb)
                nc.scalar.copy(b2_sb[:, C : 2 * C], b_sb)
                w_ps = wpsp.tile([2 * C, C], F32, tag="wps")
                nc.tensor.matmul(w_ps, b2_sb, vgT_sb, start=True, stop=True)
                nc.vector.tensor_copy(wT_sb[0:C, 0:C], w_ps[0:C, :])
                nc.scalar.copy(wT_sb[C : 2 * C, C : 2 * C],
                               w_ps[C : 2 * C, :])

            # ---- conv: stream x, y = W @ x + bias ----
            # 1 MiB bf16 granules; bias-add copies are 1024 wide over
            # 2-bank PSUM tiles, alternating DVE/ACT, writing bf16.
            CW = 1024  # copy width (PSUM tile = 2 banks)
            with tc.tile_pool(name="convps", bufs=4, space="PSUM") as cpsp:
                for n2 in range(NB // 2):
                    for gi in range(HW // GR):
                        lo = gi * GR
                        xt = xts[n2 * (HW // GR) + gi]
                        ot = op.tile([2 * C, GR], BF16, tag="ot",
                                     name=f"ot{n2}_{gi}")
                        for b in range(GR // CW):
                            ps = cpsp.tile([2 * C, CW], F32)
                            for j in range(CW // 512):
                                xsl = slice(b * CW + j * 512,
                                            b * CW + (j + 1) * 512)
                                psl = slice(j * 512, (j + 1) * 512)
                                nc.tensor.matmul(ps[:, psl], wT_sb,
                                                 xt[:, xsl],
                                                 start=True, stop=True)
                            sl = slice(b * CW, (b + 1) * CW)
                            # alternate wide bias-add copies between DVE
                            # and the ACT engine
                            if b % 2 == 0:
                                nc.vector.tensor_scalar_add(ot[:, sl], ps,
                                                            bias_sb)
                            else:
                                nc.scalar.add(ot[:, sl], ps, bias_sb)
                        # stores dispatch from the scalar engine (HWDGE,
                        # ~1us lower latency than SWDGE; ACT has slack:
                        # 2 copies + 1 dispatch = 2.8us per 5us granule).
                        # All load dispatches were issued up front on the
                        # sync ring, so no load can be blocked behind an
                        # ot sem-wait.
                        nc.scalar.dma_start(out=yv[n2, :, lo : lo + GR],
                                            in_=ot)

    nc.compile()
    return nc


_NC_CACHE = None


def _get_nc():
    global _NC_CACHE
    if _NC_CACHE is None:
        _NC_CACHE = _build()
    return _NC_CACHE


def _make_parm(z, g, bias):
    parm = np.zeros((2 * C, PCOLS), np.float32)
    parm[0:C, 0:C] = z
    parm[0:C, C] = bias
    parm[C : 2 * C, C] = bias
    parm[0:C, C + 1] = g.reshape(C)
    return parm


def _run(inputs, trace=False, **spmd_kwargs):
    nc = _get_nc()
    x = np.asarray(inputs["x"], dtype=np.float32)
    x16 = np.ascontiguousarray(x.astype(ml_dtypes.bfloat16))
    z = np.asarray(inputs["z"], dtype=np.float32)
    g = np.asarray(inputs["g"], dtype=np.float32)
    bias = np.asarray(inputs["bias"], dtype=np.float32)
    parm = _make_parm(z, g, bias)

    eye = np.eye(C, dtype=np.float32)
    in_maps = []
    for i in range(N_CORES):
        in_maps.append({"x": x16[i * NB : (i + 1) * NB], "parm": parm,
                        "eye": eye})
    res = run_bass_kernel_spmd(nc, in_maps, core_ids=list(range(N_CORES)),
                               trace=trace, **spmd_kwargs)
    out = np.concatenate([res.results[i]["out"] for i in range(N_CORES)],
                         axis=0).astype(np.float32)
    return out, res


def kernel(**inputs) -> np.ndarray:
    out, _ = _run(inputs)
    return out
